# revision 9
# baseline (speedup 1.0000x reference)
"""Trainium2 Bass kernel for nn_CMDPEncoder (VQ codebook encode + cross-batch mix + DP noise).

Problem (full shapes): base_embeddings [32,512,768] f32, codebook [8192,768] f32,
noise [32,512,768] f32, rand_idx [32,512,8] int32.
    idx[b,s]   = argmin_v ||x[b,s] - c_v||^2
    mixed[b,s] = mean_j codebook[idx[rand_idx[b,s,j], s]] + 0.1*noise[b,s]

Sharding: split the sequence dim S=512 across 8 cores (64 positions each).
rand_idx mixes across batch at the SAME position s, so every core's mixing only
needs its own quantized indices -> no collectives.

Per-core algorithm:
  1. Approximate scores q[t,v] = 2 x_t.c_v - ||c_v||^2 with a single fp16 matmul
     pass on the PE (fp16 runs 4x faster than fp32 on the tensor engine). The
     ||c||^2 bias rides along as two extra fp16 contraction rows (hi+lo split so
     the bias is fp32-accurate).
  2. Top-2 candidates per half of V via the DVE max8/max_index ops, giving 4
     candidates per token. The approximation error (~0.02) is far smaller than
     the typical top-1/top-3 score gap (~30+), so the true argmin is in this
     candidate set essentially always.
  3. Exact fp32 rescore of the 4 candidates (indirect-DMA gather of the
     codebook rows + tensor_tensor_reduce dot products), pick the true argmax.
  4. Mixing = small per-position matmul: out[(s,b),:] = sum_b' W[s][b,b'] *
     codebook[idx[b',s]], with W (counts of rand_idx / 8) precomputed on host
     from rand_idx. Gather the selected rows, one 128x128 fp32 matmul per
     4-position block, add pre-scaled noise, write out.
"""

import os
import sys
import time

import numpy as np

if "/opt/trn_rl_repo" not in sys.path:
    sys.path.insert(0, "/opt/trn_rl_repo")

B, S, D, V, K = 32, 512, 768, 8192, 8
NCORES = 8
SLOC = S // NCORES          # 64 positions per core
T = SLOC * B                # 2048 tokens per core, order t = s_local*32 + b
TT = T // 128               # 16 token tiles
DC = D // 128               # 6 contraction chunks
NHALF = 2
H = V // NHALF              # 4096 codes per half
HC = H // 512               # 8 psum chunks of 512 per half

_cached = {}


def _build_program():
    import concourse.bacc as bacc
    import concourse.mybir as mybir
    import concourse.tile as tile
    from concourse import bass

    f32 = mybir.dt.float32
    f16 = mybir.dt.float16
    u32 = mybir.dt.uint32
    u8 = mybir.dt.uint8

    nc = bacc.Bacc(
        "TRN2",
        target_bir_lowering=False,
        debug=False,
        enable_asserts=False,
        num_devices=NCORES,
    )

    xt16_d = nc.dram_tensor("xt16", [TT, DC, 128, 128], f16, kind="ExternalInput")
    ct16_d = nc.dram_tensor("ct16", [DC, NHALF, 128, H], f16, kind="ExternalInput")
    ctbias_d = nc.dram_tensor("ctbias", [2, V], f16, kind="ExternalInput")
    cb_d = nc.dram_tensor("cb", [V, D], f32, kind="ExternalInput")
    cbsq_d = nc.dram_tensor("cbsq", [V, 1], f32, kind="ExternalInput")
    x2_d = nc.dram_tensor("x2", [T, D], f32, kind="ExternalInput")
    noise_d = nc.dram_tensor("noise01", [T, D], f32, kind="ExternalInput")
    wblk_d = nc.dram_tensor("wblk", [TT, 128, 128], f32, kind="ExternalInput")
    out_d = nc.dram_tensor("out", [T, D], f32, kind="ExternalOutput")

    with tile.TileContext(nc) as tc:
        with (
            tc.tile_pool(name="ct", bufs=1) as ct_pool,
            tc.tile_pool(name="const", bufs=1) as const_pool,
            tc.tile_pool(name="xt", bufs=2) as xt_pool,
            tc.tile_pool(name="score", bufs=2) as score_pool,
            tc.tile_pool(name="gath", bufs=2) as g_pool,
            tc.tile_pool(name="io", bufs=2) as io_pool,
            tc.tile_pool(name="mix1", bufs=1) as mix_pool,
            tc.tile_pool(name="small", bufs=2) as sm_pool,
            tc.tile_pool(name="psc", bufs=4, space="PSUM") as ps_pool,
            tc.tile_pool(name="pmix", bufs=2, space="PSUM") as pm_pool,
        ):
            # resident codebook^T (fp16) + bias rows + ones
            ct_tiles = {}
            for dc in range(DC):
                for h in range(NHALF):
                    ctt = ct_pool.tile([128, H], f16, tag=f"ct{dc}h{h}")
                    nc.sync.dma_start(ctt[:], ct16_d[dc, h])
                    ct_tiles[(dc, h)] = ctt
            ctbias = const_pool.tile([2, V], f16, tag="ctbias")
            nc.sync.dma_start(ctbias[:], ctbias_d[:])
            ones2 = const_pool.tile([2, 128], f16, tag="ones2")
            nc.vector.memset(ones2[:], 1.0)

            for t in range(TT):
                # stationary X^T chunks for this token tile
                xts = []
                for dc in range(DC):
                    xtt = xt_pool.tile([128, 128], f16, tag=f"xt{dc}")
                    nc.sync.dma_start(xtt[:], xt16_d[t, dc])
                    xts.append(xtt)

                x2t = io_pool.tile([128, D], f32, tag="x2t")
                nc.sync.dma_start(x2t[:], x2_d[t * 128:(t + 1) * 128, :])

                ebest = sm_pool.tile([128, 2], f32, tag="ebest")
                ibest = sm_pool.tile([128, 2], u32, tag="ibest")
                pm = pm_pool.tile([128, D], f32, tag="pm")
                junk = g_pool.tile([128, D], f32, tag="junk")
                junkb = g_pool.tile([128, D], f32, tag="junkb")
                junk2 = g_pool.tile([128, D], f32, tag="junk2")

                for h in range(NHALF):
                    score = score_pool.tile([128, H], f32, tag="score")
                    for c in range(HC):
                        ps = ps_pool.tile([128, 512], f32, tag="ps")
                        cs = slice(c * 512, (c + 1) * 512)
                        for dc in range(DC):
                            nc.tensor.matmul(
                                ps[:],
                                lhsT=xts[dc][:],
                                rhs=ct_tiles[(dc, h)][:, cs],
                                start=(dc == 0),
                                stop=False,
                            )
                        nc.tensor.matmul(
                            ps[:],
                            lhsT=ones2[:],
                            rhs=ctbias[:, h * H + c * 512: h * H + (c + 1) * 512],
                            start=False,
                            stop=True,
                        )
                        nc.scalar.copy(out=score[:, cs], in_=ps[:])

                    vals8 = sm_pool.tile([128, 8], f32, tag="vals8")
                    idx8 = sm_pool.tile([128, 8], u32, tag="idx8")
                    nc.vector.max(out=vals8[:], in_=score[:])
                    nc.vector.max_index(out=idx8[:], in_max=vals8[:], in_values=score[:])
                    if h == 1:
                        # local half index -> global codebook index
                        nc.vector.tensor_scalar(
                            idx8[:, 0:2], idx8[:, 0:2], float(H), scalar2=None,
                            op0=mybir.AluOpType.add,
                        )

                    ga = g_pool.tile([128, D], f32, tag="ga")
                    gb = g_pool.tile([128, D], f32, tag="gb")
                    qa = sm_pool.tile([128, 1], f32, tag="qa")
                    qb = sm_pool.tile([128, 1], f32, tag="qb")
                    nc.gpsimd.indirect_dma_start(
                        out=ga[:], out_offset=None, in_=cb_d[:],
                        in_offset=bass.IndirectOffsetOnAxis(ap=idx8[:, 0:1], axis=0),
                    )
                    nc.gpsimd.indirect_dma_start(
                        out=gb[:], out_offset=None, in_=cb_d[:],
                        in_offset=bass.IndirectOffsetOnAxis(ap=idx8[:, 1:2], axis=0),
                    )
                    nc.gpsimd.indirect_dma_start(
                        out=qa[:], out_offset=None, in_=cbsq_d[:],
                        in_offset=bass.IndirectOffsetOnAxis(ap=idx8[:, 0:1], axis=0),
                    )
                    nc.gpsimd.indirect_dma_start(
                        out=qb[:], out_offset=None, in_=cbsq_d[:],
                        in_offset=bass.IndirectOffsetOnAxis(ap=idx8[:, 1:2], axis=0),
                    )

                    da = sm_pool.tile([128, 1], f32, tag="da")
                    db = sm_pool.tile([128, 1], f32, tag="db")
                    # dot products: DVE elementwise mult, ScalarE accumulating copy
                    # (tensor_tensor_reduce crashes TRN2 hw here, so avoid it)
                    nc.gpsimd.tensor_tensor(junk[:], x2t[:], ga[:], op=mybir.AluOpType.mult)
                    nc.scalar.activation(out=junk2[:], in_=junk[:],
                                         func=mybir.ActivationFunctionType.Copy,
                                         accum_out=da[:])
                    nc.gpsimd.tensor_tensor(junkb[:], x2t[:], gb[:], op=mybir.AluOpType.mult)
                    nc.scalar.activation(out=junk2[:], in_=junkb[:],
                                         func=mybir.ActivationFunctionType.Copy,
                                         accum_out=db[:])
                    # exact scores e = 2x.g - ||g||^2
                    ea = sm_pool.tile([128, 1], f32, tag="ea")
                    eb = sm_pool.tile([128, 1], f32, tag="eb")
                    nc.vector.tensor_tensor(ea[:], da[:], qa[:], op=mybir.AluOpType.subtract)
                    nc.vector.tensor_tensor(eb[:], db[:], qb[:], op=mybir.AluOpType.subtract)
                    cmp = sm_pool.tile([128, 1], u8, tag="cmp")
                    nc.vector.tensor_tensor(cmp[:], eb[:], ea[:], op=mybir.AluOpType.is_gt)
                    nc.vector.tensor_copy(ebest[:, h:h + 1], ea[:])
                    nc.vector.copy_predicated(ebest[:, h:h + 1], cmp[:], eb[:])
                    nc.vector.tensor_copy(ibest[:, h:h + 1], idx8[:, 0:1])
                    nc.vector.copy_predicated(ibest[:, h:h + 1], cmp[:], idx8[:, 1:2])

                # final winner across halves
                cmpf = sm_pool.tile([128, 1], u8, tag="cmpf")
                nc.vector.tensor_tensor(
                    cmpf[:], ebest[:, 1:2], ebest[:, 0:1], op=mybir.AluOpType.is_gt
                )
                idxf = sm_pool.tile([128, 1], u32, tag="idxf")
                nc.vector.tensor_copy(idxf[:], ibest[:, 0:1])
                nc.vector.copy_predicated(idxf[:], cmpf[:], ibest[:, 1:2])

                gf = g_pool.tile([128, D], f32, tag="gf")
                nc.gpsimd.indirect_dma_start(
                    out=gf[:], out_offset=None, in_=cb_d[:],
                    in_offset=bass.IndirectOffsetOnAxis(ap=idxf[:, 0:1], axis=0),
                )

                # mixing matmul: out[(s,b),:] = sum_{(s,b')} W[(s,b'),(s,b)] * gf[(s,b'),:]
                wt = io_pool.tile([128, 128], f32, tag="wt")
                nc.sync.dma_start(wt[:], wblk_d[t])
                nc.tensor.matmul(pm[:, 0:512], lhsT=wt[:], rhs=gf[:, 0:512],
                                 start=True, stop=True)
                nc.tensor.matmul(pm[:, 512:768], lhsT=wt[:], rhs=gf[:, 512:768],
                                 start=True, stop=True)

                nt = io_pool.tile([128, D], f32, tag="nt")
                nc.sync.dma_start(nt[:], noise_d[t * 128:(t + 1) * 128, :])
                ot = io_pool.tile([128, D], f32, tag="ot")
                mixs = mix_pool.tile([128, D], f32, tag="mixs")
                nc.scalar.copy(out=mixs[:], in_=pm[:])
                nc.gpsimd.tensor_tensor(ot[:], mixs[:], nt[:], op=mybir.AluOpType.add)
                nc.sync.dma_start(out_d[t * 128:(t + 1) * 128, :], ot[:])

    nc.compile()
    return nc


def _host_prep(base_embeddings, codebook, noise, rand_idx):
    cb = np.ascontiguousarray(codebook, dtype=np.float32)
    cbsq = np.einsum("vd,vd->v", cb.astype(np.float64), cb.astype(np.float64))
    cbsq = cbsq.astype(np.float32)

    # codebook^T fp16 tiles [DC, NHALF, 128, H]
    ct = np.ascontiguousarray(cb.T).astype(np.float16)          # [D, V]
    ct16 = ct.reshape(DC, 128, NHALF, H).transpose(0, 2, 1, 3)  # [DC,NHALF,128,H]
    ct16 = np.ascontiguousarray(ct16)

    b32 = (-cbsq).astype(np.float32)
    b_hi = b32.astype(np.float16)
    b_lo = (b32 - b_hi.astype(np.float32)).astype(np.float16)
    ctbias = np.ascontiguousarray(np.stack([b_hi, b_lo], axis=0))  # [2, V]

    cbsq2 = np.ascontiguousarray(cbsq.reshape(V, 1))

    per_core = []
    for c in range(NCORES):
        s0 = c * SLOC
        xs = base_embeddings[:, s0:s0 + SLOC, :]
        xtok = np.ascontiguousarray(xs.transpose(1, 0, 2)).reshape(T, D)
        x2 = np.ascontiguousarray(2.0 * xtok).astype(np.float32)
        xt = np.ascontiguousarray(x2.T).astype(np.float16)      # [D, T], rows = 2x
        xt16 = xt.reshape(DC, 128, TT, 128).transpose(2, 0, 1, 3)  # [TT,DC,128,128]
        xt16 = np.ascontiguousarray(xt16)

        noise01 = np.ascontiguousarray(
            (0.1 * noise[:, s0:s0 + SLOC, :]).transpose(1, 0, 2)
        ).reshape(T, D).astype(np.float32)

        ri = rand_idx[:, s0:s0 + SLOC, :]                       # [B, SLOC, K]
        counts = np.zeros((SLOC, B, B), np.float32)
        sidx = np.broadcast_to(np.arange(SLOC)[None, :, None], ri.shape).ravel()
        bidx = np.broadcast_to(np.arange(B)[:, None, None], ri.shape).ravel()
        np.add.at(counts, (sidx, bidx, np.asarray(ri).ravel()), 1.0)
        Wf = counts / K                                          # [s, b_out, b_src]
        # wblk[t][(sr,b_src),(sr,b_out)] = W[t*4+sr][b_out, b_src]
        wblk = np.zeros((TT, 128, 128), np.float32)
        for sr in range(4):
            blkW = Wf.reshape(TT, 4, B, B)[:, sr]                # [TT, b_out, b_src]
            wblk[:, sr * 32:(sr + 1) * 32, sr * 32:(sr + 1) * 32] = (
                blkW.transpose(0, 2, 1)
            )

        per_core.append({
            "xt16": xt16,
            "ct16": ct16,
            "ctbias": ctbias,
            "cb": cb,
            "cbsq": cbsq2,
            "x2": x2,
            "noise01": noise01,
            "wblk": np.ascontiguousarray(wblk),
        })
    return per_core


def kernel(base_embeddings, codebook, noise, rand_idx):
    from concourse import bass_utils

    base_embeddings = np.asarray(base_embeddings, dtype=np.float32)
    codebook = np.asarray(codebook, dtype=np.float32)
    noise = np.asarray(noise, dtype=np.float32)
    rand_idx = np.asarray(rand_idx, dtype=np.int32)

    if "nc" not in _cached:
        _cached["nc"] = _build_program()
    nc = _cached["nc"]

    in_maps = _host_prep(base_embeddings, codebook, noise, rand_idx)
    res = None
    last_exc = None
    for attempt in range(3):
        try:
            res = bass_utils.run_bass_kernel_spmd(
                nc, in_maps, core_ids=list(range(NCORES)),
                trace=bool(int(os.environ.get("KERNEL_TRACE", "0"))),
            )
            break
        except Exception as exc:  # transient wedged-device / tunnel errors
            last_exc = exc
            time.sleep(15)
    if res is None:
        raise last_exc
    _cached["last_results"] = res

    out = np.zeros((B, S, D), np.float32)
    for c in range(NCORES):
        oc = res.results[c]["out"].reshape(SLOC, B, D)
        out[:, c * SLOC:(c + 1) * SLOC, :] = oc.transpose(1, 0, 2)
    return out


# revision 10
# speedup vs baseline: 1.5711x; 1.5711x over previous
"""Trainium2 Bass kernel for nn_CMDPEncoder (VQ codebook encode + cross-batch mix + DP noise).

Problem (full shapes): base_embeddings [32,512,768] f32, codebook [8192,768] f32,
noise [32,512,768] f32, rand_idx [32,512,8] int32.
    idx[b,s]   = argmin_v ||x[b,s] - c_v||^2
    mixed[b,s] = mean_j codebook[idx[rand_idx[b,s,j], s]] + 0.1*noise[b,s]

Sharding: split the sequence dim S=512 across 8 cores (64 positions each).
rand_idx mixes across batch at the SAME position s, so every core's mixing only
needs its own quantized indices -> no collectives.

Per-core algorithm:
  1. Approximate scores q[t,v] = 2 x_t.c_v - ||c_v||^2 with a single fp16 matmul
     pass on the PE (fp16 runs 4x faster than fp32 on the tensor engine). The
     ||c||^2 bias rides along as two extra fp16 contraction rows (hi+lo split so
     the bias is fp32-accurate).
  2. Top-2 candidates per half of V via the DVE max8/max_index ops, giving 4
     candidates per token. The approximation error (~0.02) is far smaller than
     the typical top-1/top-3 score gap (~30+), so the true argmin is in this
     candidate set essentially always.
  3. Exact fp32 rescore of the 4 candidates (indirect-DMA gather of the
     codebook rows + tensor_tensor_reduce dot products), pick the true argmax.
  4. Mixing = small per-position matmul: out[(s,b),:] = sum_b' W[s][b,b'] *
     codebook[idx[b',s]], with W (counts of rand_idx / 8) precomputed on host
     from rand_idx. Gather the selected rows, one 128x128 fp32 matmul per
     4-position block, add pre-scaled noise, write out.
"""

import os
import sys
import time

import numpy as np

if "/opt/trn_rl_repo" not in sys.path:
    sys.path.insert(0, "/opt/trn_rl_repo")

B, S, D, V, K = 32, 512, 768, 8192, 8
NCORES = 8
SLOC = S // NCORES          # 64 positions per core
T = SLOC * B                # 2048 tokens per core, order t = s_local*32 + b
TT = T // 128               # 16 token tiles
DC = D // 128               # 6 contraction chunks
NHALF = 2
H = V // NHALF              # 4096 codes per half
HC = H // 512               # 8 psum chunks of 512 per half

_cached = {}


def _build_program():
    import concourse.bacc as bacc
    import concourse.mybir as mybir
    import concourse.tile as tile
    from concourse import bass

    f32 = mybir.dt.float32
    f16 = mybir.dt.float16
    u32 = mybir.dt.uint32
    u8 = mybir.dt.uint8

    nc = bacc.Bacc(
        "TRN2",
        target_bir_lowering=False,
        debug=False,
        enable_asserts=False,
        num_devices=NCORES,
    )

    xt16_d = nc.dram_tensor("xt16", [TT, DC, 128, 128], f16, kind="ExternalInput")
    ct16_d = nc.dram_tensor("ct16", [DC, NHALF, 128, H], f16, kind="ExternalInput")
    ctbias_d = nc.dram_tensor("ctbias", [2, V], f16, kind="ExternalInput")
    cb_d = nc.dram_tensor("cb", [V, D], f32, kind="ExternalInput")
    cbsq_d = nc.dram_tensor("cbsq", [V, 1], f32, kind="ExternalInput")
    x2_d = nc.dram_tensor("x2", [T, D], f32, kind="ExternalInput")
    noise_d = nc.dram_tensor("noise01", [T, D], f32, kind="ExternalInput")
    wblk_d = nc.dram_tensor("wblk", [TT, 128, 128], f32, kind="ExternalInput")
    out_d = nc.dram_tensor("out", [T, D], f32, kind="ExternalOutput")

    with tile.TileContext(nc) as tc:
        with (
            tc.tile_pool(name="ct", bufs=1) as ct_pool,
            tc.tile_pool(name="const", bufs=1) as const_pool,
            tc.tile_pool(name="xt", bufs=2) as xt_pool,
            tc.tile_pool(name="score", bufs=2) as score_pool,
            tc.tile_pool(name="gath", bufs=2) as g_pool,
            tc.tile_pool(name="io", bufs=2) as io_pool,
            tc.tile_pool(name="small", bufs=2) as sm_pool,
            tc.tile_pool(name="psc", bufs=4, space="PSUM") as ps_pool,
            tc.tile_pool(name="pmix", bufs=2, space="PSUM") as pm_pool,
        ):
            # resident codebook^T (fp16) + bias rows + ones
            ct_tiles = {}
            for dc in range(DC):
                for h in range(NHALF):
                    ctt = ct_pool.tile([128, H], f16, tag=f"ct{dc}h{h}")
                    nc.sync.dma_start(ctt[:], ct16_d[dc, h])
                    ct_tiles[(dc, h)] = ctt
            ctbias = const_pool.tile([2, V], f16, tag="ctbias")
            nc.sync.dma_start(ctbias[:], ctbias_d[:])
            ones2 = const_pool.tile([2, 128], f16, tag="ones2")
            nc.vector.memset(ones2[:], 1.0)

            for t in range(TT):
                # stationary X^T chunks for this token tile
                xts = []
                for dc in range(DC):
                    xtt = xt_pool.tile([128, 128], f16, tag=f"xt{dc}")
                    nc.sync.dma_start(xtt[:], xt16_d[t, dc])
                    xts.append(xtt)

                x2t = io_pool.tile([128, D], f32, tag="x2t")
                nc.sync.dma_start(x2t[:], x2_d[t * 128:(t + 1) * 128, :])

                ebest = sm_pool.tile([128, 2], f32, tag="ebest")
                ibest = sm_pool.tile([128, 2], u32, tag="ibest")
                pm = pm_pool.tile([128, D], f32, tag="pm")
                junk = g_pool.tile([128, D], f32, tag="junk")
                junkb = g_pool.tile([128, D], f32, tag="junkb")
                junk2 = g_pool.tile([128, D], f32, tag="junk2")

                for h in range(NHALF):
                    score = score_pool.tile([128, H], f32, tag="score")
                    for c in range(HC):
                        ps = ps_pool.tile([128, 512], f32, tag="ps")
                        cs = slice(c * 512, (c + 1) * 512)
                        for dc in range(DC):
                            nc.tensor.matmul(
                                ps[:],
                                lhsT=xts[dc][:],
                                rhs=ct_tiles[(dc, h)][:, cs],
                                start=(dc == 0),
                                stop=False,
                            )
                        nc.tensor.matmul(
                            ps[:],
                            lhsT=ones2[:],
                            rhs=ctbias[:, h * H + c * 512: h * H + (c + 1) * 512],
                            start=False,
                            stop=True,
                        )
                        nc.scalar.copy(out=score[:, cs], in_=ps[:])

                    vals8 = sm_pool.tile([128, 8], f32, tag="vals8")
                    idx8 = sm_pool.tile([128, 8], u32, tag="idx8")
                    nc.vector.max(out=vals8[:], in_=score[:])
                    nc.vector.max_index(out=idx8[:], in_max=vals8[:], in_values=score[:])
                    if h == 1:
                        # local half index -> global codebook index
                        nc.vector.tensor_scalar(
                            idx8[:, 0:2], idx8[:, 0:2], float(H), scalar2=None,
                            op0=mybir.AluOpType.add,
                        )

                    ga = g_pool.tile([128, D], f32, tag="ga")
                    gb = g_pool.tile([128, D], f32, tag="gb")
                    qa = sm_pool.tile([128, 1], f32, tag="qa")
                    qb = sm_pool.tile([128, 1], f32, tag="qb")
                    nc.gpsimd.indirect_dma_start(
                        out=ga[:], out_offset=None, in_=cb_d[:],
                        in_offset=bass.IndirectOffsetOnAxis(ap=idx8[:, 0:1], axis=0),
                    )
                    nc.gpsimd.indirect_dma_start(
                        out=gb[:], out_offset=None, in_=cb_d[:],
                        in_offset=bass.IndirectOffsetOnAxis(ap=idx8[:, 1:2], axis=0),
                    )
                    nc.gpsimd.indirect_dma_start(
                        out=qa[:], out_offset=None, in_=cbsq_d[:],
                        in_offset=bass.IndirectOffsetOnAxis(ap=idx8[:, 0:1], axis=0),
                    )
                    nc.gpsimd.indirect_dma_start(
                        out=qb[:], out_offset=None, in_=cbsq_d[:],
                        in_offset=bass.IndirectOffsetOnAxis(ap=idx8[:, 1:2], axis=0),
                    )

                    da = sm_pool.tile([128, 1], f32, tag="da")
                    db = sm_pool.tile([128, 1], f32, tag="db")
                    # dot products: DVE elementwise mult, ScalarE accumulating copy
                    # (tensor_tensor_reduce crashes TRN2 hw here, so avoid it)
                    nc.vector.tensor_tensor(junk[:], x2t[:], ga[:], op=mybir.AluOpType.mult)
                    nc.scalar.activation(out=junk2[:], in_=junk[:],
                                         func=mybir.ActivationFunctionType.Copy,
                                         accum_out=da[:])
                    nc.vector.tensor_tensor(junkb[:], x2t[:], gb[:], op=mybir.AluOpType.mult)
                    nc.scalar.activation(out=junk2[:], in_=junkb[:],
                                         func=mybir.ActivationFunctionType.Copy,
                                         accum_out=db[:])
                    # exact scores e = 2x.g - ||g||^2
                    ea = sm_pool.tile([128, 1], f32, tag="ea")
                    eb = sm_pool.tile([128, 1], f32, tag="eb")
                    nc.vector.tensor_tensor(ea[:], da[:], qa[:], op=mybir.AluOpType.subtract)
                    nc.vector.tensor_tensor(eb[:], db[:], qb[:], op=mybir.AluOpType.subtract)
                    cmp = sm_pool.tile([128, 1], u8, tag="cmp")
                    nc.vector.tensor_tensor(cmp[:], eb[:], ea[:], op=mybir.AluOpType.is_gt)
                    nc.vector.tensor_copy(ebest[:, h:h + 1], ea[:])
                    nc.vector.copy_predicated(ebest[:, h:h + 1], cmp[:], eb[:])
                    nc.vector.tensor_copy(ibest[:, h:h + 1], idx8[:, 0:1])
                    nc.vector.copy_predicated(ibest[:, h:h + 1], cmp[:], idx8[:, 1:2])

                # final winner across halves
                cmpf = sm_pool.tile([128, 1], u8, tag="cmpf")
                nc.vector.tensor_tensor(
                    cmpf[:], ebest[:, 1:2], ebest[:, 0:1], op=mybir.AluOpType.is_gt
                )
                idxf = sm_pool.tile([128, 1], u32, tag="idxf")
                nc.vector.tensor_copy(idxf[:], ibest[:, 0:1])
                nc.vector.copy_predicated(idxf[:], cmpf[:], ibest[:, 1:2])

                gf = g_pool.tile([128, D], f32, tag="gf")
                nc.gpsimd.indirect_dma_start(
                    out=gf[:], out_offset=None, in_=cb_d[:],
                    in_offset=bass.IndirectOffsetOnAxis(ap=idxf[:, 0:1], axis=0),
                )

                # mixing matmul: out[(s,b),:] = sum_{(s,b')} W[(s,b'),(s,b)] * gf[(s,b'),:]
                wt = io_pool.tile([128, 128], f32, tag="wt")
                nc.sync.dma_start(wt[:], wblk_d[t])
                nc.tensor.matmul(pm[:, 0:512], lhsT=wt[:], rhs=gf[:, 0:512],
                                 start=True, stop=True)
                nc.tensor.matmul(pm[:, 512:768], lhsT=wt[:], rhs=gf[:, 512:768],
                                 start=True, stop=True)

                nt = io_pool.tile([128, D], f32, tag="nt")
                nc.sync.dma_start(nt[:], noise_d[t * 128:(t + 1) * 128, :])
                ot = io_pool.tile([128, D], f32, tag="ot")
                nc.vector.tensor_tensor(ot[:], pm[:], nt[:], op=mybir.AluOpType.add)
                nc.sync.dma_start(out_d[t * 128:(t + 1) * 128, :], ot[:])

    nc.compile()
    return nc


def _host_prep(base_embeddings, codebook, noise, rand_idx):
    cb = np.ascontiguousarray(codebook, dtype=np.float32)
    cbsq = np.einsum("vd,vd->v", cb.astype(np.float64), cb.astype(np.float64))
    cbsq = cbsq.astype(np.float32)

    # codebook^T fp16 tiles [DC, NHALF, 128, H]
    ct = np.ascontiguousarray(cb.T).astype(np.float16)          # [D, V]
    ct16 = ct.reshape(DC, 128, NHALF, H).transpose(0, 2, 1, 3)  # [DC,NHALF,128,H]
    ct16 = np.ascontiguousarray(ct16)

    b32 = (-cbsq).astype(np.float32)
    b_hi = b32.astype(np.float16)
    b_lo = (b32 - b_hi.astype(np.float32)).astype(np.float16)
    ctbias = np.ascontiguousarray(np.stack([b_hi, b_lo], axis=0))  # [2, V]

    cbsq2 = np.ascontiguousarray(cbsq.reshape(V, 1))

    per_core = []
    for c in range(NCORES):
        s0 = c * SLOC
        xs = base_embeddings[:, s0:s0 + SLOC, :]
        xtok = np.ascontiguousarray(xs.transpose(1, 0, 2)).reshape(T, D)
        x2 = np.ascontiguousarray(2.0 * xtok).astype(np.float32)
        xt = np.ascontiguousarray(x2.T).astype(np.float16)      # [D, T], rows = 2x
        xt16 = xt.reshape(DC, 128, TT, 128).transpose(2, 0, 1, 3)  # [TT,DC,128,128]
        xt16 = np.ascontiguousarray(xt16)

        noise01 = np.ascontiguousarray(
            (0.1 * noise[:, s0:s0 + SLOC, :]).transpose(1, 0, 2)
        ).reshape(T, D).astype(np.float32)

        ri = rand_idx[:, s0:s0 + SLOC, :]                       # [B, SLOC, K]
        counts = np.zeros((SLOC, B, B), np.float32)
        sidx = np.broadcast_to(np.arange(SLOC)[None, :, None], ri.shape).ravel()
        bidx = np.broadcast_to(np.arange(B)[:, None, None], ri.shape).ravel()
        np.add.at(counts, (sidx, bidx, np.asarray(ri).ravel()), 1.0)
        Wf = counts / K                                          # [s, b_out, b_src]
        # wblk[t][(sr,b_src),(sr,b_out)] = W[t*4+sr][b_out, b_src]
        wblk = np.zeros((TT, 128, 128), np.float32)
        for sr in range(4):
            blkW = Wf.reshape(TT, 4, B, B)[:, sr]                # [TT, b_out, b_src]
            wblk[:, sr * 32:(sr + 1) * 32, sr * 32:(sr + 1) * 32] = (
                blkW.transpose(0, 2, 1)
            )

        per_core.append({
            "xt16": xt16,
            "ct16": ct16,
            "ctbias": ctbias,
            "cb": cb,
            "cbsq": cbsq2,
            "x2": x2,
            "noise01": noise01,
            "wblk": np.ascontiguousarray(wblk),
        })
    return per_core


def kernel(base_embeddings, codebook, noise, rand_idx):
    from concourse import bass_utils

    base_embeddings = np.asarray(base_embeddings, dtype=np.float32)
    codebook = np.asarray(codebook, dtype=np.float32)
    noise = np.asarray(noise, dtype=np.float32)
    rand_idx = np.asarray(rand_idx, dtype=np.int32)

    if "nc" not in _cached:
        _cached["nc"] = _build_program()
    nc = _cached["nc"]

    in_maps = _host_prep(base_embeddings, codebook, noise, rand_idx)
    res = None
    last_exc = None
    for attempt in range(3):
        try:
            res = bass_utils.run_bass_kernel_spmd(
                nc, in_maps, core_ids=list(range(NCORES)),
                trace=bool(int(os.environ.get("KERNEL_TRACE", "0"))),
            )
            break
        except Exception as exc:  # transient wedged-device / tunnel errors
            last_exc = exc
            time.sleep(15)
    if res is None:
        raise last_exc
    _cached["last_results"] = res

    out = np.zeros((B, S, D), np.float32)
    for c in range(NCORES):
        oc = res.results[c]["out"].reshape(SLOC, B, D)
        out[:, c * SLOC:(c + 1) * SLOC, :] = oc.transpose(1, 0, 2)
    return out
